# revision 1
# baseline (speedup 1.0000x reference)
"""Causal self-attention with RoPE + XSA (self-value subtraction), Trainium2.

Sharding: tensor-parallel over heads. 16 heads / 8 cores = 2 heads per core.
Each core computes QKV for its 2 heads (full batch), flash-style causal
attention in S^T layout (k on partitions, q on free dim), and a partial
output projection over its 128 feature columns. Host sums the 8 partials.

All matmuls run in float32r (full-rate fp32, ~1.6e-4 rel err on HW).

Layout notes (per core, per batch b):
  A_q, A_k : [128, 2048]  q^T/k^T, rows 0..63 = head h0 dims, 64..127 = h1
  VT       : [128, 2048]  v^T, same row layout (no RoPE)
  vext     : [128, 16, 2, 65] v tok-major per 128-tok tile per head + ones col
  attention: S^T[k, q] = matmul(lhsT=K^T[d, kc*128:], rhs=Q^T[d, qj*512:])
             P = exp(S^T / 8); V-matmul out^T[d(+denom), q] with ones column
  XSA      : strict mask (k<q) zeroes diag+future; diag exp added to the
             denominator via a tiny K=2 matmul from elementwise q.k products.
"""

import sys

if "/opt/trn_rl_repo" not in sys.path:
    sys.path.insert(0, "/opt/trn_rl_repo")

import numpy as np

B, T, D, H = 4, 2048, 1024, 16
DH = D // H  # 64
HALF = DH // 2  # 32
NCORES = 8
HPC = H // NCORES  # 2 heads per core
N = B * T  # 8192
QC = 512  # q chunk
KC = 128  # k chunk
NQJ = T // QC  # 4 q chunks per (b)
FC = D // 128  # 8 feature chunks
TC = T // QC  # 4 tok chunks per b


def _build():
    import concourse.bass as bass
    import concourse.mybir as mybir
    import concourse.tile as tile
    from concourse import bacc

    F32 = mybir.dt.float32
    F32R = mybir.dt.float32r
    AF = mybir.ActivationFunctionType
    ALU = mybir.AluOpType
    ds, ts = bass.ds, bass.ts

    nc = bacc.Bacc("TRN2")

    xT_d = nc.dram_tensor("xT", (D, N), F32, kind="ExternalInput")
    cosr_d = nc.dram_tensor("cosr", (128, T), F32, kind="ExternalInput")
    sinr_d = nc.dram_tensor("sinr", (128, T), F32, kind="ExternalInput")
    wqk_d = nc.dram_tensor("wqkT", (D, 384), F32, kind="ExternalInput")
    wp_d = nc.dram_tensor("wpT", (128, D), F32, kind="ExternalInput")
    esel_d = nc.dram_tensor("esel", (128, 2), F32, kind="ExternalInput")
    lsel_d = nc.dram_tensor("lsel", (2, 2, 65), F32, kind="ExternalInput")
    strictu_d = nc.dram_tensor("strictu", (128, 128), F32, kind="ExternalInput")
    ident_d = nc.dram_tensor("ident", (128, 128), F32, kind="ExternalInput")
    ones_d = nc.dram_tensor("ones", (128, 64), F32, kind="ExternalInput")
    out_d = nc.dram_tensor("outp", (N, D), F32, kind="ExternalOutput")

    with tile.TileContext(nc) as tc:
        with (
            tc.tile_pool(name="p1", bufs=1) as p1,
            tc.tile_pool(name="p2", bufs=2) as p2,
            tc.tile_pool(name="pxt", bufs=10) as pxt,
            tc.tile_pool(name="ppt", bufs=4) as ppt,
            tc.tile_pool(name="psA", bufs=4, space="PSUM") as psA,
            tc.tile_pool(name="psO", bufs=4, space="PSUM") as psO,
        ):
            # --- persistent weights / constants ---
            wqk_sb = p1.tile([128, FC, 384], F32R, tag="wqk")
            nc.sync.dma_start(
                wqk_sb[:], wqk_d[:].rearrange("(o p) m -> p o m", p=128).bitcast(F32R)
            )
            wp_sb = p1.tile([128, D], F32R, tag="wp")
            nc.sync.dma_start(wp_sb[:], wp_d[:].bitcast(F32R))
            cosr = p1.tile([128, T], F32, tag="cosr")
            nc.sync.dma_start(cosr[:], cosr_d[:])
            sinr = p1.tile([128, T], F32, tag="sinr")
            nc.sync.dma_start(sinr[:], sinr_d[:])
            esel_sb = p1.tile([128, 2], F32R, tag="esel")
            nc.sync.dma_start(esel_sb[:], esel_d[:].bitcast(F32R))
            lsel_sb = p1.tile([2, 2, 65], F32R, tag="lsel")
            nc.sync.dma_start(lsel_sb[:], lsel_d[:].bitcast(F32R))
            strictu = p1.tile([128, 128], F32R, tag="strictu")
            nc.sync.dma_start(strictu[:], strictu_d[:].bitcast(F32R))
            ident = p1.tile([128, 128], F32, tag="ident")
            nc.sync.dma_start(ident[:], ident_d[:])
            onesb = p1.tile([65, 64], F32R, tag="onesb")
            nc.sync.dma_start(onesb[64:65, :], ones_d[0:1, :].bitcast(F32R))

            def emit_qkv(b):
                tok0 = b * T
                A_q = p2.tile([128, T], F32R, tag="A_q", name=f"A_q{b}")
                A_k = p2.tile([128, T], F32R, tag="A_k", name=f"A_k{b}")
                VT = p2.tile([128, T], F32, tag="VT", name=f"VT{b}")
                qkp = p2.tile([128, T], F32R, tag="qkp", name=f"qkp{b}")
                vext = p2.tile([128, T // 128, 2, 65], F32R, tag="vext",
                               name=f"vext{b}")
                nc.sync.dma_start(
                    vext[:, :, :, 64],
                    ones_d[:, 0:32].rearrange("p (a c) -> p a c", c=2).bitcast(F32R),
                )
                dsts = [A_q, A_k, VT]
                for tci in range(TC):
                    tcs = ds(tci * QC, QC)
                    xts = []
                    for fc in range(FC):
                        xt = pxt.tile([128, QC], F32R, tag="xt", name=f"xt{fc}")
                        nc.sync.dma_start(
                            xt[:],
                            xT_d[ts(fc, 128), ds(tok0 + tci * QC, QC)].bitcast(F32R),
                        )
                        xts.append(xt)
                    for mi in range(3):
                        pq = psA.tile([128, QC], F32, tag="pst", name=f"pq{mi}")
                        for fc in range(FC):
                            nc.tensor.matmul(
                                pq[:],
                                wqk_sb[:, fc, ts(mi, 128)],
                                xts[fc][:],
                                start=(fc == 0),
                                stop=(fc == FC - 1),
                            )
                        nc.vector.tensor_copy(dsts[mi][:, tcs], pq[:])
                    # RoPE on this token chunk
                    for A in (A_q, A_k):
                        Bt = p2.tile([128, QC], F32R, tag="Bt")
                        nc.sync.dma_start(Bt[0:32, :], A[32:64, tcs])
                        nc.sync.dma_start(Bt[32:64, :], A[0:32, tcs])
                        nc.sync.dma_start(Bt[64:96, :], A[96:128, tcs])
                        nc.sync.dma_start(Bt[96:128, :], A[64:96, tcs])
                        nc.vector.tensor_tensor(A[:, tcs], A[:, tcs],
                                                cosr[:, tcs], ALU.mult)
                        nc.vector.tensor_tensor(Bt[:], Bt[:], sinr[:, tcs], ALU.mult)
                        nc.vector.tensor_tensor(A[:, tcs], A[:, tcs], Bt[:], ALU.add)
                    nc.vector.tensor_tensor(qkp[:, tcs], A_q[:, tcs], A_k[:, tcs],
                                            ALU.mult)
                    # V token-major via PE transpose for this chunk
                    for tt in range(4 * tci, 4 * tci + 4):
                        ptr = psA.tile([128, 128], F32, tag="pst", name="ptr")
                        nc.tensor.transpose(ptr[:], VT[:, ts(tt, 128)], ident[:])
                        nc.vector.tensor_copy(vext[:, tt, 0, 0:64], ptr[:, 0:64])
                        nc.vector.tensor_copy(vext[:, tt, 1, 0:64], ptr[:, 64:128])
                return b, A_q, A_k, qkp, vext

            def emit_attn(st):
                b, A_q, A_k, qkp, vext = st
                outT = p2.tile([128, T], F32R, tag="outT", name=f"outT{b}")
                oT1 = p2.tile([64, T], F32R, tag="oT1", name=f"oT1{b}")
                for qj in range(NQJ):
                    q0 = qj * QC
                    pd = psA.tile([2, QC], F32, tag="pst", name="pd")
                    nc.tensor.matmul(
                        pd[:], esel_sb[:], qkp[:, ds(q0, QC)], start=True, stop=True
                    )
                    de = p2.tile([2, QC], F32R, tag="de")
                    nc.scalar.activation(de[:], pd[:], AF.Exp, scale=0.125)

                    po = [
                        psO.tile([65, QC], F32, tag="po65", name=f"po{h}")
                        for h in range(2)
                    ]
                    nkc = 4 * qj + 4
                    for kc in range(nkc):
                        o = kc - 4 * qj
                        c0 = 128 * o if o > 0 else 0
                        psts = []
                        for h in range(2):
                            r0 = 64 * h
                            pst = psA.tile([128, QC], F32, tag="pst",
                                           name=f"pst{h}")
                            nc.tensor.matmul(
                                pst[:, c0:QC],
                                A_k[r0 : r0 + 64, ts(kc, 128)],
                                A_q[r0 : r0 + 64, ds(q0 + c0, QC - c0)],
                                start=True,
                                stop=True,
                            )
                            psts.append(pst)
                        for h in range(2):
                            pt = ppt.tile([128, QC], F32R, tag="pt", name=f"pt{h}")
                            nc.scalar.activation(
                                pt[:, c0:QC], psts[h][:, c0:QC], AF.Exp, scale=0.125
                            )
                            if o >= 0:
                                nc.vector.tensor_tensor(
                                    pt[:, ds(c0, 128)],
                                    pt[:, ds(c0, 128)],
                                    strictu[:],
                                    ALU.mult,
                                )
                            nc.tensor.matmul(
                                po[h][:, c0:QC],
                                vext[:, kc, h, :],
                                pt[:, c0:QC],
                                start=(kc == 0),
                                stop=False,
                            )
                    for h in range(2):
                        nc.tensor.matmul(
                            po[h][:], lsel_sb[:, h, :], de[:], start=False, stop=True
                        )
                        dnr = p2.tile([65, QC], F32R, tag="dnr")
                        nc.scalar.copy(dnr[64:65, :], po[h][64:65, :])
                        pb = psA.tile([64, QC], F32, tag="pst", name="pb")
                        nc.tensor.matmul(
                            pb[:], onesb[64:65, :], dnr[64:65, :],
                            start=True, stop=True,
                        )
                        bc = p2.tile([64, QC], F32, tag="bc")
                        nc.vector.reciprocal_approx_fast(bc[:], pb[:])
                        dst = outT[0:64, ds(q0, QC)] if h == 0 else oT1[:, ds(q0, QC)]
                        nc.vector.tensor_tensor(dst, po[h][0:64, :], bc[:], ALU.mult)

                nc.sync.dma_start(outT[64:128, :], oT1[:])
                return outT

            def emit_proj(b, outT):
                tok0 = b * T
                for mt in range(T // 128):
                    for nj in range(2):
                        pp = psA.tile([128, 512], F32, tag="pst", name="pp")
                        nc.tensor.matmul(
                            pp[:],
                            outT[:, ts(mt, 128)],
                            wp_sb[:, ts(nj, 512)],
                            start=True,
                            stop=True,
                        )
                        po_sb = p2.tile([128, 512], F32, tag="po_sb")
                        nc.vector.tensor_copy(po_sb[:], pp[:])
                        nc.sync.dma_start(
                            out_d[ds(tok0 + mt * 128, 128), ts(nj, 512)], po_sb[:]
                        )

            sts = emit_qkv(0)
            for b in range(B):
                cur = sts
                if b + 1 < B:
                    sts = emit_qkv(b + 1)
                outT = emit_attn(cur)
                emit_proj(b, outT)

    nc.finalize()
    return nc


def _host_inputs(x, cos, sin, W_qkv, W_proj):
    """Build per-core input maps."""
    x = np.asarray(x, dtype=np.float32)
    cos = np.asarray(cos, dtype=np.float32)
    sin = np.asarray(sin, dtype=np.float32)
    W_qkv = np.asarray(W_qkv, dtype=np.float32)
    W_proj = np.asarray(W_proj, dtype=np.float32)

    xT = np.ascontiguousarray(x.reshape(N, D).T)  # [D, N]
    cosT = np.ascontiguousarray(cos[0, 0].T)  # [32, T]
    sinT = np.ascontiguousarray(sin[0, 0].T)
    cosr = np.tile(cosT, (4, 1))  # [128, T]
    sinr = np.concatenate([-sinT, sinT, -sinT, sinT], axis=0)  # [128, T]

    esel = np.zeros((128, 2), np.float32)
    esel[0:64, 0] = 1.0
    esel[64:128, 1] = 1.0
    lsel = np.zeros((2, 2, 65), np.float32)
    lsel[0, 0, 64] = 1.0  # head 0: row 0 -> out row 64
    lsel[1, 1, 64] = 1.0
    strictu = np.triu(np.ones((128, 128), np.float32), 1)  # 1 iff k < q
    ident = np.eye(128, dtype=np.float32)
    ones = np.ones((128, 64), np.float32)

    in_maps = []
    for c in range(NCORES):
        h0, h1 = 2 * c, 2 * c + 1
        cols = []
        for base in (0, D, 2 * D):  # q, k, v row blocks of W_qkv
            cols.append(W_qkv[base + 64 * h0 : base + 64 * h0 + 64])
            cols.append(W_qkv[base + 64 * h1 : base + 64 * h1 + 64])
        wqkT = np.ascontiguousarray(np.concatenate(cols, axis=0).T)  # [D, 384]
        wpT = np.ascontiguousarray(W_proj[:, 128 * c : 128 * c + 128].T)  # [128, D]
        in_maps.append(
            {
                "xT": xT,
                "cosr": cosr,
                "sinr": sinr,
                "wqkT": wqkT,
                "wpT": wpT,
                "esel": esel,
                "lsel": lsel,
                "strictu": strictu,
                "ident": ident,
                "ones": ones,
            }
        )
    return in_maps


_NC_CACHE = {}


def _get_nc():
    if "nc" not in _NC_CACHE:
        _NC_CACHE["nc"] = _build()
    return _NC_CACHE["nc"]


def kernel(x, cos, sin, W_qkv, W_proj, _trace=False, _trace_cores=None):
    from concourse import bass_utils

    nc = _get_nc()
    in_maps = _host_inputs(x, cos, sin, W_qkv, W_proj)
    res = bass_utils.run_bass_kernel_spmd(
        nc,
        in_maps,
        core_ids=list(range(NCORES)),
        trace=_trace,
        trace_cores=_trace_cores,
    )
    out = np.zeros((N, D), np.float64)
    for r in res.results:
        out += r["outp"].astype(np.float64)
    ret = out.astype(np.float32).reshape(B, T, D)
    kernel.last_results = res
    return ret



# revision 12
# speedup vs baseline: 1.8270x; 1.8270x over previous
"""Causal self-attention with RoPE + XSA (self-value subtraction), Trainium2.

Sharding: hybrid batch x head-group. Core c -> (b = c//2, gg = c%2), i.e.
each core owns one batch and 8 of the 16 heads (4 head-pairs). Each core:
  - computes QKV for its 8 heads over its batch (full D contraction),
  - flash-style causal attention in S^T layout per head-pair,
  - partial output projection over its 512 input features.
Host sums the 2 partials per batch. This cuts per-core HBM traffic ~4x vs
pure head sharding (x read 4.2MB, out write 8.4MB per core).

Engine assignment (per-core):
  PE    : QKV / S^T / P@V / diag-sel / denom-broadcast / proj matmuls (bf16)
  ACT   : exp(S/8) into bf16 pt tiles, exp of diagonal q.k
  DVE   : psum->sbuf q/k copies, RoPE muls, strict-causal mask, final divide
  gpsimd: V copies, vext re-striding, denominator-row copies, proj copies
  DMA   : HBM loads/stores, RoPE half-swaps, V transposes (XBAR), oT1 hop

Layout notes (per core, per head-pair p with heads h0, h1):
  A_q, A_k : [128, 2048] bf16 q^T/k^T; rows 0..63 = h0 dims, 64..127 = h1
  VT       : [128, 2048] bf16 v^T, same row layout
  vext     : [128, 16, 2, 65] bf16: per 128-token tile per head, token-major
             V (cols 0..63) + ones column (col 64) for the softmax denom.
             Built by XBAR dma transpose (packed) + strided gpsimd copy.
  attention: S^T[k, q] = matmul(lhsT=K^T[dh, kc*128:], rhs=Q^T[dh, qj*512:])
             P = exp(S^T/8) bf16; V-matmul out^T[d(+denom), q], M=65
  XSA      : strict mask (k<q) zeroes diag+future in P; diag exp added to
             the denominator via a K=4 select matmul, then one DVE divide.
"""

import sys

if "/opt/trn_rl_repo" not in sys.path:
    sys.path.insert(0, "/opt/trn_rl_repo")

import numpy as np
import ml_dtypes

BF = ml_dtypes.bfloat16

B, T, D, H = 4, 2048, 1024, 16
DH = D // H  # 64
HALF = DH // 2  # 32
NCORES = 8
NP = 4  # head-pairs per core
QC = 512  # q chunk
NQJ = T // QC  # 4
FC = D // 128  # 8 feature chunks
TC = T // QC  # 4 token chunks


def _build():
    import concourse.bass as bass
    import concourse.mybir as mybir
    import concourse.tile as tile
    from concourse import bacc

    F32 = mybir.dt.float32
    F32R = mybir.dt.float32r
    BF16 = mybir.dt.bfloat16
    AF = mybir.ActivationFunctionType
    ALU = mybir.AluOpType
    ds, ts = bass.ds, bass.ts

    nc = bacc.Bacc("TRN2")

    xT_d = nc.dram_tensor("xT", (D, T), BF16, kind="ExternalInput")
    cosr_d = nc.dram_tensor("cosr", (128, T), BF16, kind="ExternalInput")
    sinr_d = nc.dram_tensor("sinr", (128, T), BF16, kind="ExternalInput")
    wqk_d = nc.dram_tensor("wqkT", (D, 3 * 512), BF16, kind="ExternalInput")
    wp_d = nc.dram_tensor("wpT", (512, D), BF16, kind="ExternalInput")
    esel_d = nc.dram_tensor("esel", (128, 2), BF16, kind="ExternalInput")
    lsel_d = nc.dram_tensor("lsel", (2, 2, 65), F32, kind="ExternalInput")
    onesf_d = nc.dram_tensor("onesf", (1, 64), F32, kind="ExternalInput")
    strictu_d = nc.dram_tensor("strictu", (128, 128), BF16, kind="ExternalInput")
    ones_d = nc.dram_tensor("ones", (128, 64), BF16, kind="ExternalInput")
    out_d = nc.dram_tensor("outp", (T, D), F32, kind="ExternalOutput")

    with tile.TileContext(nc) as tc:
        with (
            tc.tile_pool(name="p1", bufs=1) as p1,
            tc.tile_pool(name="pA", bufs=2) as pA,
            tc.tile_pool(name="pOT", bufs=4) as pOT,
            tc.tile_pool(name="pw", bufs=2) as pw,
            tc.tile_pool(name="ppt", bufs=4) as ppt,
            tc.tile_pool(name="psQ", bufs=3, space="PSUM") as psQ,
            tc.tile_pool(name="psS", bufs=3, space="PSUM") as psS,
            tc.tile_pool(name="psO", bufs=2, space="PSUM") as psO,
        ):
            # --- persistent weights / constants / x ---
            wqk_sb = p1.tile([128, FC, 3 * 512], BF16, tag="wqk")
            wqk_r = wqk_d[:].rearrange("(o p) m -> p o m", p=128)
            for fc in range(FC):
                nc.sync.dma_start(wqk_sb[:, fc, :], wqk_r[:, fc, :])
            wp_sb = p1.tile([128, 4, D], BF16, tag="wp")
            nc.sync.dma_start(wp_sb[:], wp_d[:].rearrange("(o p) m -> p o m", p=128))
            xT_sb = p1.tile([128, FC, T], BF16, tag="xT")
            xT_r = xT_d[:].rearrange("(o p) t -> p o t", p=128)
            for fc in range(FC):
                nc.sync.dma_start(xT_sb[:, fc, :], xT_r[:, fc, :])
            cosr = p1.tile([128, T], BF16, tag="cosr")
            nc.sync.dma_start(cosr[:], cosr_d[:])
            sinr = p1.tile([128, T], BF16, tag="sinr")
            nc.sync.dma_start(sinr[:], sinr_d[:])
            esel_sb = p1.tile([128, 2], BF16, tag="esel")
            nc.sync.dma_start(esel_sb[:], esel_d[:])
            lsel_sb = p1.tile([2, 2, 65], F32R, tag="lsel")
            nc.sync.dma_start(lsel_sb[:], lsel_d[:].bitcast(F32R))
            onesf_sb = p1.tile([65, 64], F32R, tag="onesf")
            nc.sync.dma_start(onesf_sb[64:65, :], onesf_d[:].bitcast(F32R))
            strictu = p1.tile([128, 128], BF16, tag="strictu")
            nc.sync.dma_start(strictu[:], strictu_d[:])

            outTs = [None] * NP

            def emit_qkv(p):
                A_q = pA.tile([128, T], BF16, tag="A_q", name=f"A_q{p}")
                A_k = pA.tile([128, T], BF16, tag="A_k", name=f"A_k{p}")
                VT = pA.tile([128, T], BF16, tag="VT", name=f"VT{p}")
                vext = pA.tile([128, 16, 2, 65], BF16, tag="vext", name=f"vext{p}")
                nc.sync.dma_start(
                    vext[:, :, :, 64],
                    ones_d[:, 0:32].rearrange("p (a c) -> p a c", c=2),
                )
                dsts = [A_q, A_k]
                for tci in range(TC):
                    tcs = ds(tci * QC, QC)
                    for mi in range(3):
                        pq = psQ.tile([128, QC], F32, tag="psq", name=f"pq{mi}")
                        for fc in range(FC):
                            nc.tensor.matmul(
                                pq[:],
                                wqk_sb[:, fc, ds(p * 384 + mi * 128, 128)],
                                xT_sb[:, fc, tcs],
                                start=(fc == 0),
                                stop=(fc == FC - 1),
                            )
                        if mi == 2:
                            nc.vector.tensor_copy(VT[:, tcs], pq[:])
                        else:
                            nc.vector.tensor_copy(dsts[mi][:, tcs], pq[:])
                    # RoPE on this token chunk
                    for A in (A_q, A_k):
                        Bt = pw.tile([128, QC], BF16, tag="Bt")
                        nc.sync.dma_start(Bt[0:32, :], A[32:64, tcs])
                        nc.sync.dma_start(Bt[32:64, :], A[0:32, tcs])
                        nc.sync.dma_start(Bt[64:96, :], A[96:128, tcs])
                        nc.sync.dma_start(Bt[96:128, :], A[64:96, tcs])
                        nc.vector.tensor_tensor(A[:, tcs], A[:, tcs],
                                                cosr[:, tcs], ALU.mult)
                        nc.vector.tensor_tensor(Bt[:], Bt[:], sinr[:, tcs], ALU.mult)
                        nc.vector.tensor_tensor(A[:, tcs], A[:, tcs], Bt[:], ALU.add)
                for h in range(2):
                    vtr = pw.tile([128, 16, 64], BF16, tag="vtr", name=f"vtr{h}")
                    nc.sync.dma_start_transpose(vtr[:], VT[ts(h, 64), :])
                    nc.gpsimd.tensor_copy(vext[:, :, h, 0:64], vtr[:])
                return p, A_q, A_k, vext

            def emit_proj_chunk(qj):
                for mt in range(4 * qj, 4 * qj + 4):
                    for nj in range(2):
                        pp = psQ.tile([128, 512], F32, tag="psq", name="pp")
                        for p in range(NP):
                            nc.tensor.matmul(
                                pp[:],
                                outTs[p][:, ts(mt, 128)],
                                wp_sb[:, p, ts(nj, 512)],
                                start=(p == 0),
                                stop=(p == NP - 1),
                            )
                        ob = pw.tile([128, 512], F32, tag="ob")
                        nc.vector.tensor_copy(ob[:], pp[:])
                        nc.sync.dma_start(
                            out_d[ds(mt * 128, 128), ts(nj, 512)], ob[:]
                        )

            def emit_attn(st, last):
                p, A_q, A_k, vext = st
                outT = pOT.tile([128, T], BF16, tag="outT", name=f"outT{p}")
                outTs[p] = outT
                for qj in range(NQJ):
                    q0 = qj * QC
                    qw = ds(q0, QC)
                    qkp = pw.tile([128, QC], BF16, tag="qkp")
                    nc.vector.tensor_tensor(qkp[:], A_q[:, qw], A_k[:, qw],
                                            ALU.mult)
                    pd = psQ.tile([2, QC], F32, tag="psq", name="pd")
                    nc.tensor.matmul(pd[:], esel_sb[:], qkp[:], start=True,
                                     stop=True)
                    de = pw.tile([2, QC], F32R, tag="de")
                    nc.scalar.activation(de[:], pd[:], AF.Exp, scale=0.125)

                    po = [
                        psO.tile([65, QC], F32, tag="po", name=f"po{h}")
                        for h in range(2)
                    ]
                    nkc = 4 * qj + 4
                    for kc in range(nkc):
                        o = kc - 4 * qj
                        c0 = 128 * o if o > 0 else 0
                        for h in range(2):
                            r0 = 64 * h
                            pst = psS.tile([128, QC], F32, tag="pst",
                                           name=f"pst{h}")
                            nc.tensor.matmul(
                                pst[:, c0:QC],
                                A_k[r0 : r0 + 64, ts(kc, 128)],
                                A_q[r0 : r0 + 64, ds(q0 + c0, QC - c0)],
                                start=True,
                                stop=True,
                            )
                            pt = ppt.tile([128, QC], BF16, tag="pt",
                                          name=f"pt{h}")
                            nc.scalar.activation(
                                pt[:, c0:QC], pst[:, c0:QC], AF.Exp, scale=0.125
                            )
                            if o >= 0:
                                nc.vector.tensor_tensor(
                                    pt[:, ds(c0, 128)],
                                    pt[:, ds(c0, 128)],
                                    strictu[:],
                                    ALU.mult,
                                )
                            nc.tensor.matmul(
                                po[h][:, c0:QC],
                                vext[:, kc, h, :],
                                pt[:, c0:QC],
                                start=(kc == 0),
                                stop=False,
                            )
                    for h in range(2):
                        nc.tensor.matmul(
                            po[h][:], lsel_sb[:, h, :], de[:],
                            start=False, stop=True,
                        )
                        dnr = pw.tile([65, QC], F32R, tag="dnr")
                        nc.vector.tensor_copy(dnr[64:65, :], po[h][64:65, :])
                        pb = psQ.tile([64, QC], F32, tag="psq", name="pb")
                        nc.tensor.matmul(
                            pb[:], onesf_sb[64:65, :], dnr[64:65, :],
                            start=True, stop=True,
                        )
                        bc = pw.tile([64, QC], F32, tag="bc")
                        nc.vector.reciprocal_approx_fast(bc[:], pb[:])
                        if h == 0:
                            nc.vector.tensor_tensor(
                                outT[0:64, qw], po[h][0:64, :], bc[:], ALU.mult
                            )
                        else:
                            oT1 = pw.tile([64, QC], BF16, tag="oT1")
                            nc.vector.tensor_tensor(
                                oT1[:], po[h][0:64, :], bc[:], ALU.mult
                            )
                            nc.sync.dma_start(outT[64:128, qw], oT1[:])
                    if last:
                        emit_proj_chunk(qj)

            st = emit_qkv(0)
            for p in range(NP):
                cur = st
                if p + 1 < NP:
                    st = emit_qkv(p + 1)
                emit_attn(cur, last=(p == NP - 1))

    nc.finalize()
    return nc


def _host_inputs(x, cos, sin, W_qkv, W_proj):
    """Build per-core input maps. Core c -> batch c//2, head-group c%2."""
    x = np.asarray(x, dtype=np.float32)
    cos = np.asarray(cos, dtype=np.float32)
    sin = np.asarray(sin, dtype=np.float32)
    W_qkv = np.asarray(W_qkv, dtype=np.float32)
    W_proj = np.asarray(W_proj, dtype=np.float32)

    cosT = np.ascontiguousarray(cos[0, 0].T)  # [32, T]
    sinT = np.ascontiguousarray(sin[0, 0].T)
    cosr = np.tile(cosT, (4, 1)).astype(BF)  # [128, T]
    sinr = np.concatenate([-sinT, sinT, -sinT, sinT], axis=0).astype(BF)

    esel = np.zeros((128, 2), BF)
    esel[0:64, 0] = 1.0
    esel[64:128, 1] = 1.0
    lsel = np.zeros((2, 2, 65), np.float32)
    lsel[0, 0, 64] = 1.0
    lsel[1, 1, 64] = 1.0
    onesf = np.ones((1, 64), np.float32)
    strictu = np.triu(np.ones((128, 128), np.float32), 1).astype(BF)
    ones = np.ones((128, 64), BF)

    xTb = [
        np.ascontiguousarray(x[b].T).astype(BF) for b in range(B)
    ]  # [D, T] per batch

    in_maps = []
    for c in range(NCORES):
        b, gg = c // 2, c % 2
        heads = [8 * gg + i for i in range(8)]
        cols = []
        for pr in range(NP):
            h0, h1 = heads[2 * pr], heads[2 * pr + 1]
            for base in (0, D, 2 * D):  # q, k, v row blocks of W_qkv
                cols.append(W_qkv[base + 64 * h0 : base + 64 * h0 + 64])
                cols.append(W_qkv[base + 64 * h1 : base + 64 * h1 + 64])
        wqkT = np.ascontiguousarray(np.concatenate(cols, axis=0).T).astype(BF)
        featc = np.concatenate(
            [np.arange(64 * h, 64 * h + 64) for h in heads]
        )
        wpT = np.ascontiguousarray(W_proj[:, featc].T).astype(BF)  # [512, D]
        in_maps.append(
            {
                "xT": xTb[b],
                "cosr": cosr,
                "sinr": sinr,
                "wqkT": wqkT,
                "wpT": wpT,
                "esel": esel,
                "lsel": lsel,
                "onesf": onesf,
                "strictu": strictu,
                "ones": ones,
            }
        )
    return in_maps


_NC_CACHE = {}


def _get_nc():
    if "nc" not in _NC_CACHE:
        _NC_CACHE["nc"] = _build()
    return _NC_CACHE["nc"]


def kernel(x, cos, sin, W_qkv, W_proj, _trace=False, _trace_cores=None):
    from concourse import bass_utils

    nc = _get_nc()
    in_maps = _host_inputs(x, cos, sin, W_qkv, W_proj)
    res = bass_utils.run_bass_kernel_spmd(
        nc,
        in_maps,
        core_ids=list(range(NCORES)),
        trace=_trace,
        trace_cores=_trace_cores,
    )
    out = np.zeros((B, T, D), np.float32)
    for c, r in enumerate(res.results):
        out[c // 2] += r["outp"]
    kernel.last_results = res
    return out


# revision 13
# speedup vs baseline: 2.0017x; 1.0956x over previous
"""Causal self-attention with RoPE + XSA (self-value subtraction), Trainium2.

Sharding: hybrid batch x head-group. Core c -> (b = c//2, gg = c%2), i.e.
each core owns one batch and 8 of the 16 heads (4 head-pairs). Each core:
  - computes QKV for its 8 heads over its batch (full D contraction),
  - flash-style causal attention in S^T layout per head-pair,
  - partial output projection over its 512 input features.
Host sums the 2 partials per batch. This cuts per-core HBM traffic ~4x vs
pure head sharding (x read 4.2MB, out write 8.4MB per core).

Engine assignment (per-core):
  PE    : QKV / S^T / P@V / diag-sel / denom-broadcast / proj matmuls (bf16)
  ACT   : exp(S/8) into bf16 pt tiles, exp of diagonal q.k
  DVE   : psum->sbuf copies, RoPE muls, reciprocal, final normalize
  gpsimd: strict-causal mask, vext re-striding, RoPE swap DMA dispatch
  DMA   : HBM loads/stores, V transposes (XBAR), oT1 hop

Scheduling: per-engine instruction order is static, so PE density is
arranged explicitly: the V matmuls lag the S matmuls by 2 k-blocks
(hiding the exp latency), and the QKV matmul groups of pair p+1 are
spliced between attention k-blocks of pair p as PE filler so the tensor
engine never idles long enough for the HAM clock gate to re-throttle.

Layout notes (per core, per head-pair p with heads h0, h1):
  A_q, A_k : [128, 2048] bf16 q^T/k^T; rows 0..63 = h0 dims, 64..127 = h1
  VT       : [128, 2048] bf16 v^T, same row layout
  vext     : [128, 16, 2, 65] bf16: per 128-token tile per head, token-major
             V (cols 0..63) + ones column (col 64) for the softmax denom.
             Built by XBAR dma transpose (packed) + strided gpsimd copy.
  attention: S^T[k, q] = matmul(lhsT=K^T[dh, kc*128:], rhs=Q^T[dh, qj*512:])
             P = exp(S^T/8) bf16; V-matmul out^T[d(+denom), q], M=65
  XSA      : strict mask (k<q) zeroes diag+future in P; diag exp added to
             the denominator row via a tiny K=2 matmul, then broadcast
             reciprocal and normalize.
"""

import sys

if "/opt/trn_rl_repo" not in sys.path:
    sys.path.insert(0, "/opt/trn_rl_repo")

import numpy as np
import ml_dtypes

BF = ml_dtypes.bfloat16

B, T, D, H = 4, 2048, 1024, 16
DH = D // H  # 64
HALF = DH // 2  # 32
NCORES = 8
NP = 4  # head-pairs per core
QC = 512  # q chunk
NQJ = T // QC  # 4
FC = D // 128  # 8 feature chunks
TC = T // QC  # 4 token chunks
VLAG = 2  # V matmuls trail S matmuls by this many k-blocks


def _build():
    import concourse.bass as bass
    import concourse.mybir as mybir
    import concourse.tile as tile
    from concourse import bacc

    F32 = mybir.dt.float32
    F32R = mybir.dt.float32r
    BF16 = mybir.dt.bfloat16
    AF = mybir.ActivationFunctionType
    ALU = mybir.AluOpType
    ds, ts = bass.ds, bass.ts

    nc = bacc.Bacc("TRN2")

    xT_d = nc.dram_tensor("xT", (D, T), BF16, kind="ExternalInput")
    cosr_d = nc.dram_tensor("cosr", (128, T), BF16, kind="ExternalInput")
    sinr_d = nc.dram_tensor("sinr", (128, T), BF16, kind="ExternalInput")
    wqk_d = nc.dram_tensor("wqkT", (D, 3 * 512), BF16, kind="ExternalInput")
    wp_d = nc.dram_tensor("wpT", (512, D), BF16, kind="ExternalInput")
    esel_d = nc.dram_tensor("esel", (128, 2), BF16, kind="ExternalInput")
    lsel_d = nc.dram_tensor("lsel", (2, 2, 65), F32, kind="ExternalInput")
    onesf_d = nc.dram_tensor("onesf", (1, 64), F32, kind="ExternalInput")
    strictu_d = nc.dram_tensor("strictu", (128, 128), BF16, kind="ExternalInput")
    ones_d = nc.dram_tensor("ones", (128, 64), BF16, kind="ExternalInput")
    out_d = nc.dram_tensor("outp", (T, D), F32, kind="ExternalOutput")

    with tile.TileContext(nc) as tc:
        with (
            tc.tile_pool(name="p1", bufs=1) as p1,
            tc.tile_pool(name="pA", bufs=2) as pA,
            tc.tile_pool(name="pOT", bufs=4) as pOT,
            tc.tile_pool(name="pw", bufs=2) as pw,
            tc.tile_pool(name="ppt", bufs=6) as ppt,
            tc.tile_pool(name="psQ", bufs=2, space="PSUM") as psQ,
            tc.tile_pool(name="psS", bufs=4, space="PSUM") as psS,
            tc.tile_pool(name="psO", bufs=2, space="PSUM") as psO,
        ):
            # --- persistent weights / constants / x (fc-interleaved so the
            # first QKV matmul group is ready as early as possible) ---
            wqk_sb = p1.tile([128, FC, 3 * 512], BF16, tag="wqk")
            wqk_r = wqk_d[:].rearrange("(o p) m -> p o m", p=128)
            xT_sb = p1.tile([128, FC, T], BF16, tag="xT")
            xT_r = xT_d[:].rearrange("(o p) t -> p o t", p=128)
            for fc in range(FC):
                nc.sync.dma_start(wqk_sb[:, fc, :], wqk_r[:, fc, :])
                nc.sync.dma_start(xT_sb[:, fc, :], xT_r[:, fc, :])
            cosr = p1.tile([128, T], BF16, tag="cosr")
            nc.sync.dma_start(cosr[:], cosr_d[:])
            sinr = p1.tile([128, T], BF16, tag="sinr")
            nc.sync.dma_start(sinr[:], sinr_d[:])
            wp_sb = p1.tile([128, 4, D], BF16, tag="wp")
            nc.sync.dma_start(wp_sb[:], wp_d[:].rearrange("(o p) m -> p o m", p=128))
            esel_sb = p1.tile([128, 2], BF16, tag="esel")
            nc.sync.dma_start(esel_sb[:], esel_d[:])
            lsel_sb = p1.tile([2, 2, 65], F32R, tag="lsel")
            nc.sync.dma_start(lsel_sb[:], lsel_d[:].bitcast(F32R))
            onesf_sb = p1.tile([65, 64], F32R, tag="onesf")
            nc.sync.dma_start(onesf_sb[64:65, :], onesf_d[:].bitcast(F32R))
            strictu = p1.tile([128, 128], BF16, tag="strictu")
            nc.sync.dma_start(strictu[:], strictu_d[:])

            outTs = [None] * NP

            def qkv_alloc(p):
                A_q = pA.tile([128, T], BF16, tag="A_q", name=f"A_q{p}")
                A_k = pA.tile([128, T], BF16, tag="A_k", name=f"A_k{p}")
                VT = pA.tile([128, T], BF16, tag="VT", name=f"VT{p}")
                vext = pA.tile([128, 16, 2, 65], BF16, tag="vext", name=f"vext{p}")
                nc.sync.dma_start(
                    vext[:, :, :, 64],
                    ones_d[:, 0:32].rearrange("p (a c) -> p a c", c=2),
                )
                return p, A_q, A_k, VT, vext

            def qkv_steps(st):
                """Generator of fine-grained QKV emission steps for pair p."""
                p, A_q, A_k, VT, vext = st
                dsts = [A_q, A_k, VT]
                for tci in range(TC):
                    tcs = ds(tci * QC, QC)
                    for mi in range(3):
                        pq = psQ.tile([128, QC], F32, tag="psq", name=f"pq{mi}")
                        for fc in range(FC):
                            nc.tensor.matmul(
                                pq[:],
                                wqk_sb[:, fc, ds(p * 384 + mi * 128, 128)],
                                xT_sb[:, fc, tcs],
                                start=(fc == 0),
                                stop=(fc == FC - 1),
                            )
                        nc.vector.tensor_copy(dsts[mi][:, tcs], pq[:])
                        yield
                    # RoPE on this token chunk (swap dispatch via gpsimd DGE)
                    for A in (A_q, A_k):
                        Bt = pw.tile([128, QC], BF16, tag="Bt")
                        nc.gpsimd.dma_start(Bt[0:32, :], A[32:64, tcs])
                        nc.gpsimd.dma_start(Bt[32:64, :], A[0:32, tcs])
                        nc.gpsimd.dma_start(Bt[64:96, :], A[96:128, tcs])
                        nc.gpsimd.dma_start(Bt[96:128, :], A[64:96, tcs])
                        nc.vector.tensor_tensor(A[:, tcs], A[:, tcs],
                                                cosr[:, tcs], ALU.mult)
                        nc.vector.tensor_tensor(Bt[:], Bt[:], sinr[:, tcs], ALU.mult)
                        nc.vector.tensor_tensor(A[:, tcs], A[:, tcs], Bt[:], ALU.add)
                        yield
                for h in range(2):
                    vtr = pw.tile([128, 16, 64], BF16, tag="vtr", name=f"vtr{h}")
                    nc.sync.dma_start_transpose(vtr[:], VT[ts(h, 64), :])
                    nc.gpsimd.tensor_copy(vext[:, :, h, 0:64], vtr[:])
                    yield

            def emit_proj_chunk(qj):
                for mt in range(4 * qj, 4 * qj + 4):
                    for nj in range(2):
                        pp = psQ.tile([128, 512], F32, tag="psq", name="pp")
                        for p in range(NP):
                            nc.tensor.matmul(
                                pp[:],
                                outTs[p][:, ts(mt, 128)],
                                wp_sb[:, p, ts(nj, 512)],
                                start=(p == 0),
                                stop=(p == NP - 1),
                            )
                        ob = pw.tile([128, 512], F32, tag="ob")
                        nc.vector.tensor_copy(ob[:], pp[:])
                        nc.sync.dma_start(
                            out_d[ds(mt * 128, 128), ts(nj, 512)], ob[:]
                        )

            def emit_attn(st, filler, last):
                """Attention for pair p. `filler` is a generator of QKV steps
                for the next pair, drained between k-blocks to keep the PE
                stream dense."""
                p, A_q, A_k, VT, vext = st

                def pull(n=1):
                    for _ in range(n):
                        next(filler, None)

                outT = pOT.tile([128, T], BF16, tag="outT", name=f"outT{p}")
                outTs[p] = outT
                for qj in range(NQJ):
                    q0 = qj * QC
                    qw = ds(q0, QC)
                    qkp = pw.tile([128, QC], BF16, tag="qkp")
                    nc.vector.tensor_tensor(qkp[:], A_q[:, qw], A_k[:, qw],
                                            ALU.mult)
                    pd = psQ.tile([2, QC], F32, tag="psq", name="pd")
                    nc.tensor.matmul(pd[:], esel_sb[:], qkp[:], start=True,
                                     stop=True)
                    de = pw.tile([2, QC], F32R, tag="de")
                    nc.scalar.activation(de[:], pd[:], AF.Exp, scale=0.125)

                    po = [
                        psO.tile([65, QC], F32, tag="po", name=f"po{h}")
                        for h in range(2)
                    ]
                    nkc = 4 * qj + 4
                    pts = {}

                    def emit_v(kc):
                        o = kc - 4 * qj
                        c0 = 128 * o if o > 0 else 0
                        for h in range(2):
                            nc.tensor.matmul(
                                po[h][:, c0:QC],
                                vext[:, kc, h, :],
                                pts.pop((kc, h))[:, c0:QC],
                                start=(kc == 0),
                                stop=False,
                            )

                    for kc in range(nkc):
                        o = kc - 4 * qj
                        c0 = 128 * o if o > 0 else 0
                        for h in range(2):
                            r0 = 64 * h
                            pst = psS.tile([128, QC], F32, tag="pst",
                                           name=f"pst{h}")
                            nc.tensor.matmul(
                                pst[:, c0:QC],
                                A_k[r0 : r0 + 64, ts(kc, 128)],
                                A_q[r0 : r0 + 64, ds(q0 + c0, QC - c0)],
                                start=True,
                                stop=True,
                            )
                            pt = ppt.tile([128, QC], BF16, tag="pt",
                                          name=f"pt{h}")
                            nc.scalar.activation(
                                pt[:, c0:QC], pst[:, c0:QC], AF.Exp, scale=0.125
                            )
                            if o >= 0:
                                nc.gpsimd.tensor_tensor(
                                    pt[:, ds(c0, 128)],
                                    pt[:, ds(c0, 128)],
                                    strictu[:],
                                    ALU.mult,
                                )
                            pts[(kc, h)] = pt
                        if kc >= VLAG:
                            emit_v(kc - VLAG)
                        if kc % 3 == 2:
                            pull()
                    for kc in range(max(0, nkc - VLAG), nkc):
                        emit_v(kc)
                    for h in range(2):
                        nc.tensor.matmul(
                            po[h][:], lsel_sb[:, h, :], de[:],
                            start=False, stop=True,
                        )
                        dnr = pw.tile([65, QC], F32R, tag="dnr")
                        nc.vector.tensor_copy(dnr[64:65, :], po[h][64:65, :])
                        pb = psQ.tile([64, QC], F32, tag="psq", name="pb")
                        nc.tensor.matmul(
                            pb[:], onesf_sb[64:65, :], dnr[64:65, :],
                            start=True, stop=True,
                        )
                        bc = pw.tile([64, QC], F32, tag="bc")
                        nc.vector.reciprocal_approx_fast(bc[:], pb[:])
                        if h == 0:
                            nc.vector.tensor_tensor(
                                outT[0:64, qw], po[h][0:64, :], bc[:], ALU.mult
                            )
                        else:
                            oT1 = pw.tile([64, QC], BF16, tag="oT1")
                            nc.vector.tensor_tensor(
                                oT1[:], po[h][0:64, :], bc[:], ALU.mult
                            )
                            nc.gpsimd.dma_start(outT[64:128, qw], oT1[:])
                    pull(2)
                    if last:
                        emit_proj_chunk(qj)
                # drain any remaining filler steps
                for _ in filler:
                    pass

            def empty_gen():
                return iter(())

            st = qkv_alloc(0)
            for _ in qkv_steps(st):
                pass
            for p in range(NP):
                cur = st
                if p + 1 < NP:
                    st = qkv_alloc(p + 1)
                    filler = qkv_steps(st)
                else:
                    filler = empty_gen()
                emit_attn(cur, filler, last=(p == NP - 1))

    nc.finalize()
    return nc


def _host_inputs(x, cos, sin, W_qkv, W_proj):
    """Build per-core input maps. Core c -> batch c//2, head-group c%2."""
    x = np.asarray(x, dtype=np.float32)
    cos = np.asarray(cos, dtype=np.float32)
    sin = np.asarray(sin, dtype=np.float32)
    W_qkv = np.asarray(W_qkv, dtype=np.float32)
    W_proj = np.asarray(W_proj, dtype=np.float32)

    cosT = np.ascontiguousarray(cos[0, 0].T)  # [32, T]
    sinT = np.ascontiguousarray(sin[0, 0].T)
    cosr = np.tile(cosT, (4, 1)).astype(BF)  # [128, T]
    sinr = np.concatenate([-sinT, sinT, -sinT, sinT], axis=0).astype(BF)

    esel = np.zeros((128, 2), BF)
    esel[0:64, 0] = 1.0
    esel[64:128, 1] = 1.0
    lsel = np.zeros((2, 2, 65), np.float32)
    lsel[0, 0, 64] = 1.0
    lsel[1, 1, 64] = 1.0
    onesf = np.ones((1, 64), np.float32)
    strictu = np.triu(np.ones((128, 128), np.float32), 1).astype(BF)
    ones = np.ones((128, 64), BF)

    xTb = [
        np.ascontiguousarray(x[b].T).astype(BF) for b in range(B)
    ]  # [D, T] per batch

    in_maps = []
    for c in range(NCORES):
        b, gg = c // 2, c % 2
        heads = [8 * gg + i for i in range(8)]
        cols = []
        for pr in range(NP):
            h0, h1 = heads[2 * pr], heads[2 * pr + 1]
            for base in (0, D, 2 * D):  # q, k, v row blocks of W_qkv
                cols.append(W_qkv[base + 64 * h0 : base + 64 * h0 + 64])
                cols.append(W_qkv[base + 64 * h1 : base + 64 * h1 + 64])
        wqkT = np.ascontiguousarray(np.concatenate(cols, axis=0).T).astype(BF)
        featc = np.concatenate(
            [np.arange(64 * h, 64 * h + 64) for h in heads]
        )
        wpT = np.ascontiguousarray(W_proj[:, featc].T).astype(BF)  # [512, D]
        in_maps.append(
            {
                "xT": xTb[b],
                "cosr": cosr,
                "sinr": sinr,
                "wqkT": wqkT,
                "wpT": wpT,
                "esel": esel,
                "lsel": lsel,
                "onesf": onesf,
                "strictu": strictu,
                "ones": ones,
            }
        )
    return in_maps


_NC_CACHE = {}


def _get_nc():
    if "nc" not in _NC_CACHE:
        _NC_CACHE["nc"] = _build()
    return _NC_CACHE["nc"]


def kernel(x, cos, sin, W_qkv, W_proj, _trace=False, _trace_cores=None):
    from concourse import bass_utils

    nc = _get_nc()
    in_maps = _host_inputs(x, cos, sin, W_qkv, W_proj)
    res = bass_utils.run_bass_kernel_spmd(
        nc,
        in_maps,
        core_ids=list(range(NCORES)),
        trace=_trace,
        trace_cores=_trace_cores,
    )
    out = np.zeros((B, T, D), np.float32)
    for c, r in enumerate(res.results):
        out[c // 2] += r["outp"]
    kernel.last_results = res
    return out


# revision 18
# speedup vs baseline: 2.0600x; 1.0291x over previous
"""Causal self-attention with RoPE + XSA (self-value subtraction), Trainium2.

Sharding: hybrid batch x head-group. Core c -> (b = c//2, gg = c%2), i.e.
each core owns one batch and 8 of the 16 heads (4 head-pairs). Each core:
  - computes QKV for its 8 heads over its batch (full D contraction),
  - flash-style causal attention in S^T layout per head-pair,
  - partial output projection over its 512 input features.
Host sums the 2 partials per batch. This cuts per-core HBM traffic ~4x vs
pure head sharding (x read 4.2MB, out write 8.4MB per core).

Engine assignment (per-core):
  PE    : QKV / S^T / P@V / diag-sel / denom-broadcast / proj matmuls (bf16)
  ACT   : exp(S/8) into bf16 pt tiles, exp of diagonal q.k
  DVE   : psum->sbuf copies, RoPE muls, reciprocal, final normalize
  gpsimd: strict-causal mask, vext re-striding, RoPE swap DMA dispatch
  DMA   : HBM loads/stores, V transposes (XBAR), oT1 hop

Scheduling: per-engine instruction order is static, so PE density is
arranged explicitly: the V matmuls lag the S matmuls by 2 k-blocks
(hiding the exp latency), and the QKV matmul groups of pair p+1 are
spliced between attention k-blocks of pair p as PE filler so the tensor
engine never idles long enough for the HAM clock gate to re-throttle.

Layout notes (per core, per head-pair p with heads h0, h1):
  A_q, A_k : [128, 2048] bf16 q^T/k^T; rows 0..63 = h0 dims, 64..127 = h1
  VT       : [128, 2048] bf16 v^T, same row layout
  vext     : [128, 16, 2, 65] bf16: per 128-token tile per head, token-major
             V (cols 0..63) + ones column (col 64) for the softmax denom.
             Built by XBAR dma transpose (packed) + strided gpsimd copy.
  attention: S^T[k, q] = matmul(lhsT=K^T[dh, kc*128:], rhs=Q^T[dh, qj*512:])
             P = exp(S^T/8) bf16; V-matmul out^T[d(+denom), q], M=65
  XSA      : strict mask (k<q) zeroes diag+future in P; diag exp added to
             the denominator row via a tiny K=2 matmul, then broadcast
             reciprocal and normalize.
"""

import sys

if "/opt/trn_rl_repo" not in sys.path:
    sys.path.insert(0, "/opt/trn_rl_repo")

import numpy as np
import ml_dtypes

BF = ml_dtypes.bfloat16

B, T, D, H = 4, 2048, 1024, 16
DH = D // H  # 64
HALF = DH // 2  # 32
NCORES = 8
NP = 4  # head-pairs per core
QC = 512  # q chunk
NQJ = T // QC  # 4
FC = D // 128  # 8 feature chunks
TC = T // QC  # 4 token chunks
VLAG = 2  # V matmuls trail S matmuls by this many k-blocks


def _build():
    import concourse.bass as bass
    import concourse.mybir as mybir
    import concourse.tile as tile
    from concourse import bacc

    F32 = mybir.dt.float32
    F32R = mybir.dt.float32r
    BF16 = mybir.dt.bfloat16
    AF = mybir.ActivationFunctionType
    ALU = mybir.AluOpType
    ds, ts = bass.ds, bass.ts

    nc = bacc.Bacc("TRN2")

    xT_d = nc.dram_tensor("xT", (D, T), BF16, kind="ExternalInput")
    cosr_d = nc.dram_tensor("cosr", (128, T), BF16, kind="ExternalInput")
    sinr_d = nc.dram_tensor("sinr", (128, T), BF16, kind="ExternalInput")
    wqk_d = nc.dram_tensor("wqkT", (D, 3 * 512), BF16, kind="ExternalInput")
    wp_d = nc.dram_tensor("wpT", (512, D), BF16, kind="ExternalInput")
    esel_d = nc.dram_tensor("esel", (128, 2), BF16, kind="ExternalInput")
    lsel_d = nc.dram_tensor("lsel", (2, 2, 65), F32, kind="ExternalInput")
    onesf_d = nc.dram_tensor("onesf", (1, 64), F32, kind="ExternalInput")
    strictu_d = nc.dram_tensor("strictu", (128, 128), BF16, kind="ExternalInput")
    ones_d = nc.dram_tensor("ones", (128, 64), BF16, kind="ExternalInput")
    out_d = nc.dram_tensor("outp", (T, D), F32, kind="ExternalOutput")

    with tile.TileContext(nc) as tc:
        with (
            tc.tile_pool(name="p1", bufs=1) as p1,
            tc.tile_pool(name="pA", bufs=2) as pA,
            tc.tile_pool(name="pOT", bufs=4) as pOT,
            tc.tile_pool(name="pw", bufs=2) as pw,
            tc.tile_pool(name="ppt", bufs=6) as ppt,
            tc.tile_pool(name="psQ", bufs=2, space="PSUM") as psQ,
            tc.tile_pool(name="psS", bufs=2, space="PSUM") as psS,
            tc.tile_pool(name="psO", bufs=2, space="PSUM") as psO,
        ):
            # --- persistent weights / constants / x (fc-interleaved so the
            # first QKV matmul group is ready as early as possible) ---
            wqk_sb = p1.tile([128, FC, 3 * 512], BF16, tag="wqk")
            wqk_r = wqk_d[:].rearrange("(o p) m -> p o m", p=128)
            xT_sb = p1.tile([128, FC, T], BF16, tag="xT")
            xT_r = xT_d[:].rearrange("(o p) t -> p o t", p=128)
            for fc in range(FC):
                nc.sync.dma_start(wqk_sb[:, fc, :], wqk_r[:, fc, :])
                nc.sync.dma_start(xT_sb[:, fc, :], xT_r[:, fc, :])
            cosr = p1.tile([128, T], BF16, tag="cosr")
            nc.sync.dma_start(cosr[:], cosr_d[:])
            sinr = p1.tile([128, T], BF16, tag="sinr")
            nc.sync.dma_start(sinr[:], sinr_d[:])
            wp_sb = p1.tile([128, 4, D], BF16, tag="wp")
            nc.sync.dma_start(wp_sb[:], wp_d[:].rearrange("(o p) m -> p o m", p=128))
            esel_sb = p1.tile([128, 2], BF16, tag="esel")
            nc.sync.dma_start(esel_sb[:], esel_d[:])
            lsel_sb = p1.tile([2, 2, 65], F32R, tag="lsel")
            nc.sync.dma_start(lsel_sb[:], lsel_d[:].bitcast(F32R))
            onesf_sb = p1.tile([65, 64], F32R, tag="onesf")
            nc.sync.dma_start(onesf_sb[64:65, :], onesf_d[:].bitcast(F32R))
            strictu = p1.tile([128, 128], BF16, tag="strictu")
            nc.sync.dma_start(strictu[:], strictu_d[:])

            outTs = [None] * NP

            def qkv_alloc(p):
                A_q = pA.tile([128, T], BF16, tag="A_q", name=f"A_q{p}")
                A_k = pA.tile([128, T], BF16, tag="A_k", name=f"A_k{p}")
                VT = pA.tile([128, T], BF16, tag="VT", name=f"VT{p}")
                vext = pA.tile([128, 16, 2, 65], BF16, tag="vext", name=f"vext{p}")
                nc.sync.dma_start(
                    vext[:, :, :, 64],
                    ones_d[:, 0:32].rearrange("p (a c) -> p a c", c=2),
                )
                return p, A_q, A_k, VT, vext

            def qkv_steps(st, first=False):
                """Generator of fine-grained QKV emission steps for pair p.
                For the first (un-overlapped) pair, psum->sbuf copies go to
                the otherwise-idle scalar engine so the vector engine's RoPE
                backlog can't stall PSUM bank recycling."""
                p, A_q, A_k, VT, vext = st
                dsts = [A_q, A_k, VT]
                for tci in range(TC):
                    tcs = ds(tci * QC, QC)
                    for mi in range(3):
                        pq = psQ.tile([128, QC], F32, tag="psq", name=f"pq{mi}")
                        for fc in range(FC):
                            nc.tensor.matmul(
                                pq[:],
                                wqk_sb[:, fc, ds(p * 384 + mi * 128, 128)],
                                xT_sb[:, fc, tcs],
                                start=(fc == 0),
                                stop=(fc == FC - 1),
                            )
                        if first:
                            nc.scalar.copy(dsts[mi][:, tcs], pq[:])
                        else:
                            nc.vector.tensor_copy(dsts[mi][:, tcs], pq[:])
                        yield
                    # RoPE on this token chunk (swap dispatch via gpsimd DGE)
                    for A in (A_q, A_k):
                        Bt = pw.tile([128, QC], BF16, tag="Bt")
                        nc.gpsimd.dma_start(Bt[0:32, :], A[32:64, tcs])
                        nc.gpsimd.dma_start(Bt[32:64, :], A[0:32, tcs])
                        nc.gpsimd.dma_start(Bt[64:96, :], A[96:128, tcs])
                        nc.gpsimd.dma_start(Bt[96:128, :], A[64:96, tcs])
                        nc.vector.tensor_tensor(A[:, tcs], A[:, tcs],
                                                cosr[:, tcs], ALU.mult)
                        nc.vector.tensor_tensor(Bt[:], Bt[:], sinr[:, tcs], ALU.mult)
                        nc.vector.tensor_tensor(A[:, tcs], A[:, tcs], Bt[:], ALU.add)
                        yield
                    # V transposes per half as soon as the half is complete,
                    # so attention's first V matmul isn't gated on a long
                    # transpose + re-stride chain at the pair seam.
                    if tci in (1, 3):
                        t0 = (tci - 1) * QC
                        for h in range(2):
                            vtr = pw.tile([128, 8, 64], BF16, tag="vtr",
                                          name=f"vtr{h}")
                            nc.sync.dma_start_transpose(
                                vtr[:], VT[ts(h, 64), ds(t0, 2 * QC)]
                            )
                            nc.gpsimd.tensor_copy(
                                vext[:, ds(8 * (tci // 2), 8), h, 0:64], vtr[:]
                            )
                            yield

            def proj_steps(qj):
                for mt in range(4 * qj, 4 * qj + 4):
                    for nj in range(2):
                        pp = psQ.tile([128, 512], F32, tag="psq", name="pp")
                        for p in range(NP):
                            nc.tensor.matmul(
                                pp[:],
                                outTs[p][:, ts(mt, 128)],
                                wp_sb[:, p, ts(nj, 512)],
                                start=(p == 0),
                                stop=(p == NP - 1),
                            )
                        ob = pw.tile([128, 512], F32, tag="ob")
                        nc.vector.tensor_copy(ob[:], pp[:])
                        nc.sync.dma_start(
                            out_d[ds(mt * 128, 128), ts(nj, 512)], ob[:]
                        )
                        yield

            def emit_attn(st, fillq, last):
                """Attention for pair p. `fillq` is a list of generators of
                filler steps (QKV of the next pair, or the projection of
                completed token chunks), drained between k-blocks to keep
                the PE stream dense."""
                p, A_q, A_k, VT, vext = st

                def pull(n=1):
                    for _ in range(n):
                        while fillq:
                            try:
                                next(fillq[0])
                                break
                            except StopIteration:
                                fillq.pop(0)

                outT = pOT.tile([128, T], BF16, tag="outT", name=f"outT{p}")
                outTs[p] = outT
                for qj in range(NQJ):
                    q0 = qj * QC
                    qw = ds(q0, QC)
                    qkp = pw.tile([128, QC], BF16, tag="qkp")
                    nc.vector.tensor_tensor(qkp[:], A_q[:, qw], A_k[:, qw],
                                            ALU.mult)
                    pd = psQ.tile([2, QC], F32, tag="psq", name="pd")
                    nc.tensor.matmul(pd[:], esel_sb[:], qkp[:], start=True,
                                     stop=True)
                    de = pw.tile([2, QC], F32R, tag="de")
                    nc.scalar.activation(de[:], pd[:], AF.Exp, scale=0.125)

                    po = [
                        psO.tile([65, QC], F32, tag="po", name=f"po{h}")
                        for h in range(2)
                    ]
                    nkc = 4 * qj + 4
                    pts = {}

                    def emit_v(kc):
                        o = kc - 4 * qj
                        c0 = 128 * o if o > 0 else 0
                        pt = pts.pop(kc)
                        for h in range(2):
                            nc.tensor.matmul(
                                po[h][:, c0:QC],
                                vext[:, kc, h, :],
                                pt[:, h, c0:QC],
                                start=(kc == 0),
                                stop=False,
                            )

                    for kc in range(nkc):
                        o = kc - 4 * qj
                        c0 = 128 * o if o > 0 else 0
                        # both heads' S^T blocks into one 2-bank psum tile
                        pst = psS.tile([128, 2, QC], F32, tag="pst")
                        for h in range(2):
                            r0 = 64 * h
                            nc.tensor.matmul(
                                pst[:, h, c0:QC],
                                A_k[r0 : r0 + 64, ts(kc, 128)],
                                A_q[r0 : r0 + 64, ds(q0 + c0, QC - c0)],
                                start=True,
                                stop=True,
                            )
                        # one exp for both heads
                        pt = ppt.tile([128, 2, QC], BF16, tag="pt")
                        nc.scalar.activation(
                            pt[:, :, c0:QC], pst[:, :, c0:QC], AF.Exp,
                            scale=0.125,
                        )
                        if o >= 0:
                            for h in range(2):
                                nc.gpsimd.tensor_tensor(
                                    pt[:, h, ds(c0, 128)],
                                    pt[:, h, ds(c0, 128)],
                                    strictu[:],
                                    ALU.mult,
                                )
                        pts[kc] = pt
                        if kc >= VLAG:
                            emit_v(kc - VLAG)
                        if kc % 3 == 2:
                            pull()
                    for kc in range(max(0, nkc - VLAG), nkc):
                        emit_v(kc)
                    for h in range(2):
                        nc.tensor.matmul(
                            po[h][:], lsel_sb[:, h, :], de[:],
                            start=False, stop=True,
                        )
                        dnr = pw.tile([65, QC], F32R, tag="dnr")
                        nc.vector.tensor_copy(dnr[64:65, :], po[h][64:65, :])
                        pb = psQ.tile([64, QC], F32, tag="psq", name="pb")
                        nc.tensor.matmul(
                            pb[:], onesf_sb[64:65, :], dnr[64:65, :],
                            start=True, stop=True,
                        )
                        bc = pw.tile([64, QC], F32, tag="bc")
                        nc.vector.reciprocal_approx_fast(bc[:], pb[:])
                        if h == 0:
                            nc.vector.tensor_tensor(
                                outT[0:64, qw], po[h][0:64, :], bc[:], ALU.mult
                            )
                        else:
                            oT1 = pw.tile([64, QC], BF16, tag="oT1")
                            nc.vector.tensor_tensor(
                                oT1[:], po[h][0:64, :], bc[:], ALU.mult
                            )
                            nc.gpsimd.dma_start(outT[64:128, qw], oT1[:])
                    pull(2)
                    if last:
                        fillq.append(proj_steps(qj))
                # drain any remaining filler steps
                while fillq:
                    for _ in fillq.pop(0):
                        pass

            st = qkv_alloc(0)
            for _ in qkv_steps(st, first=True):
                pass
            for p in range(NP):
                cur = st
                if p + 1 < NP:
                    st = qkv_alloc(p + 1)
                    fillq = [qkv_steps(st)]
                else:
                    fillq = []
                emit_attn(cur, fillq, last=(p == NP - 1))

    nc.finalize()
    return nc


def _host_inputs(x, cos, sin, W_qkv, W_proj):
    """Build per-core input maps. Core c -> batch c//2, head-group c%2."""
    x = np.asarray(x, dtype=np.float32)
    cos = np.asarray(cos, dtype=np.float32)
    sin = np.asarray(sin, dtype=np.float32)
    W_qkv = np.asarray(W_qkv, dtype=np.float32)
    W_proj = np.asarray(W_proj, dtype=np.float32)

    cosT = np.ascontiguousarray(cos[0, 0].T)  # [32, T]
    sinT = np.ascontiguousarray(sin[0, 0].T)
    cosr = np.tile(cosT, (4, 1)).astype(BF)  # [128, T]
    sinr = np.concatenate([-sinT, sinT, -sinT, sinT], axis=0).astype(BF)

    esel = np.zeros((128, 2), BF)
    esel[0:64, 0] = 1.0
    esel[64:128, 1] = 1.0
    lsel = np.zeros((2, 2, 65), np.float32)
    lsel[0, 0, 64] = 1.0
    lsel[1, 1, 64] = 1.0
    onesf = np.ones((1, 64), np.float32)
    strictu = np.triu(np.ones((128, 128), np.float32), 1).astype(BF)
    ones = np.ones((128, 64), BF)

    xTb = [
        np.ascontiguousarray(x[b].T).astype(BF) for b in range(B)
    ]  # [D, T] per batch

    in_maps = []
    for c in range(NCORES):
        b, gg = c // 2, c % 2
        heads = [8 * gg + i for i in range(8)]
        cols = []
        for pr in range(NP):
            h0, h1 = heads[2 * pr], heads[2 * pr + 1]
            for base in (0, D, 2 * D):  # q, k, v row blocks of W_qkv
                cols.append(W_qkv[base + 64 * h0 : base + 64 * h0 + 64])
                cols.append(W_qkv[base + 64 * h1 : base + 64 * h1 + 64])
        wqkT = np.ascontiguousarray(np.concatenate(cols, axis=0).T).astype(BF)
        featc = np.concatenate(
            [np.arange(64 * h, 64 * h + 64) for h in heads]
        )
        wpT = np.ascontiguousarray(W_proj[:, featc].T).astype(BF)  # [512, D]
        in_maps.append(
            {
                "xT": xTb[b],
                "cosr": cosr,
                "sinr": sinr,
                "wqkT": wqkT,
                "wpT": wpT,
                "esel": esel,
                "lsel": lsel,
                "onesf": onesf,
                "strictu": strictu,
                "ones": ones,
            }
        )
    return in_maps


_NC_CACHE = {}


def _get_nc():
    if "nc" not in _NC_CACHE:
        _NC_CACHE["nc"] = _build()
    return _NC_CACHE["nc"]


def kernel(x, cos, sin, W_qkv, W_proj, _trace=False, _trace_cores=None):
    from concourse import bass_utils

    nc = _get_nc()
    in_maps = _host_inputs(x, cos, sin, W_qkv, W_proj)
    res = bass_utils.run_bass_kernel_spmd(
        nc,
        in_maps,
        core_ids=list(range(NCORES)),
        trace=_trace,
        trace_cores=_trace_cores,
    )
    out = np.zeros((B, T, D), np.float32)
    for c, r in enumerate(res.results):
        out[c // 2] += r["outp"]
    kernel.last_results = res
    return out


# revision 27
# speedup vs baseline: 2.3533x; 1.1424x over previous
"""Causal self-attention with RoPE + XSA (self-value subtraction), Trainium2.

Sharding: hybrid batch x head-group. Core c -> (b = c//2, gg = c%2), i.e.
each core owns one batch and 8 of the 16 heads (4 head-pairs). Each core:
  - computes QKV for its 8 heads over its batch (full D contraction),
  - flash-style causal attention in S^T layout per head-pair,
  - partial output projection over its 512 input features.
Host sums the 2 partials per batch. This cuts per-core HBM traffic ~4x vs
pure head sharding (x read 4.2MB, out write 8.4MB per core).

Engine assignment (per-core):
  PE    : QKV / S^T / P@V / diag-sel / denom-broadcast / proj matmuls (bf16)
  ACT   : exp(S/8) into bf16 pt tiles, exp of diagonal q.k
  DVE   : psum->sbuf copies, RoPE muls, reciprocal, final normalize
  gpsimd: strict-causal mask, vext re-striding, RoPE swap DMA dispatch
  DMA   : HBM loads/stores, V transposes (XBAR), oT1 hop

Scheduling: per-engine instruction order is static, so PE density is
arranged explicitly: the V matmuls lag the S matmuls by 2 k-blocks
(hiding the exp latency), and the QKV matmul groups of pair p+1 are
spliced between attention k-blocks of pair p as PE filler so the tensor
engine never idles long enough for the HAM clock gate to re-throttle.

Layout notes (per core, per head-pair p with heads h0, h1):
  A_q, A_k : [128, 2048] bf16 q^T/k^T; rows 0..63 = h0 dims, 64..127 = h1
  VT       : [128, 2048] bf16 v^T, same row layout
  vext     : [128, 16, 2, 65] bf16: per 128-token tile per head, token-major
             V (cols 0..63) + ones column (col 64) for the softmax denom.
             Built by XBAR dma transpose (packed) + strided gpsimd copy.
  attention: S^T[k, q] = matmul(lhsT=K^T[dh, kc*128:], rhs=Q^T[dh, qj*512:])
             P = exp(S^T/8) bf16; V-matmul out^T[d(+denom), q], M=65
  XSA      : strict mask (k<q) zeroes diag+future in P; diag exp added to
             the denominator row via a tiny K=2 matmul, then broadcast
             reciprocal and normalize.
"""

import sys

if "/opt/trn_rl_repo" not in sys.path:
    sys.path.insert(0, "/opt/trn_rl_repo")

import numpy as np
import ml_dtypes

BF = ml_dtypes.bfloat16

B, T, D, H = 4, 2048, 1024, 16
DH = D // H  # 64
HALF = DH // 2  # 32
NCORES = 8
NP = 4  # head-pairs per core
QC = 512  # q chunk
NQJ = T // QC  # 4
FC = D // 128  # 8 feature chunks
TC = T // QC  # 4 token chunks
VLAG = 2  # V matmuls trail S matmuls by this many k-blocks


def _build():
    import concourse.bass as bass
    import concourse.mybir as mybir
    import concourse.tile as tile
    from concourse import bacc

    F32 = mybir.dt.float32
    F32R = mybir.dt.float32r
    BF16 = mybir.dt.bfloat16
    AF = mybir.ActivationFunctionType
    ALU = mybir.AluOpType
    ds, ts = bass.ds, bass.ts

    nc = bacc.Bacc("TRN2")

    xT_d = nc.dram_tensor("xT", (D, T), BF16, kind="ExternalInput")
    cosr_d = nc.dram_tensor("cosr", (128, T), BF16, kind="ExternalInput")
    sinr_d = nc.dram_tensor("sinr", (128, T), BF16, kind="ExternalInput")
    wqk_d = nc.dram_tensor("wqkT", (D, 3 * 512), BF16, kind="ExternalInput")
    wp_d = nc.dram_tensor("wpT", (512, D), BF16, kind="ExternalInput")
    esel_d = nc.dram_tensor("esel", (128, 2), BF16, kind="ExternalInput")
    lsel_d = nc.dram_tensor("lsel", (2, 2, 65), F32, kind="ExternalInput")
    onesf_d = nc.dram_tensor("onesf", (1, 64), F32, kind="ExternalInput")
    strictu_d = nc.dram_tensor("strictu", (128, 128), BF16, kind="ExternalInput")
    out_d = nc.dram_tensor("outp", (T, D), F32, kind="ExternalOutput")

    with tile.TileContext(nc) as tc:
        with (
            tc.tile_pool(name="p1", bufs=1) as p1,
            tc.tile_pool(name="pA", bufs=2) as pA,
            tc.tile_pool(name="pOT", bufs=4) as pOT,
            tc.tile_pool(name="pw", bufs=2) as pw,
            tc.tile_pool(name="ppt", bufs=6) as ppt,
            tc.tile_pool(name="psQ", bufs=2, space="PSUM") as psQ,
            tc.tile_pool(name="psS", bufs=2, space="PSUM") as psS,
            tc.tile_pool(name="psO", bufs=2, space="PSUM") as psO,
        ):
            # --- persistent weights / constants / x. Load order is tuned so
            # pair-0's first QKV matmul group (needs wqk pair-0 cols + the
            # first token chunk of x across all feature chunks) is ready
            # after ~1.8 MB instead of the full 7.3 MB input set. ---
            wqk_sb = p1.tile([128, FC, 3 * 512], BF16, tag="wqk")
            wqk_r = wqk_d[:].rearrange("(o p) m -> p o m", p=128)
            xT_sb = p1.tile([128, FC, T], BF16, tag="xT")
            xT_r = xT_d[:].rearrange("(o p) t -> p o t", p=128)
            for fc in range(FC):
                nc.sync.dma_start(
                    wqk_sb[:, fc, ds(0, 384)], wqk_r[:, fc, ds(0, 384)]
                )
                nc.sync.dma_start(
                    xT_sb[:, fc, ds(0, QC)], xT_r[:, fc, ds(0, QC)]
                )
            cosr = p1.tile([128, T], BF16, tag="cosr")
            nc.sync.dma_start(cosr[:], cosr_d[:])
            sinr = p1.tile([128, T], BF16, tag="sinr")
            nc.sync.dma_start(sinr[:], sinr_d[:])
            esel_sb = p1.tile([128, 2], BF16, tag="esel")
            nc.sync.dma_start(esel_sb[:], esel_d[:])
            lsel_sb = p1.tile([2, 2, 65], F32R, tag="lsel")
            nc.sync.dma_start(lsel_sb[:], lsel_d[:].bitcast(F32R))
            onesf_sb = p1.tile([65, 64], F32R, tag="onesf")
            nc.sync.dma_start(onesf_sb[64:65, :], onesf_d[:].bitcast(F32R))
            strictu = p1.tile([128, 128], BF16, tag="strictu")
            nc.sync.dma_start(strictu[:], strictu_d[:])
            for tci in range(1, TC):
                for fc in range(FC):
                    nc.sync.dma_start(
                        xT_sb[:, fc, ds(tci * QC, QC)],
                        xT_r[:, fc, ds(tci * QC, QC)],
                    )
                pr = tci  # stream in the next pair's weight columns
                for fc in range(FC):
                    nc.sync.dma_start(
                        wqk_sb[:, fc, ds(pr * 384, 384)],
                        wqk_r[:, fc, ds(pr * 384, 384)],
                    )
            wp_sb = p1.tile([128, 4, D], BF16, tag="wp")
            nc.sync.dma_start(wp_sb[:], wp_d[:].rearrange("(o p) m -> p o m", p=128))

            outTs = [None] * NP

            def qkv_alloc(p):
                A_q = pA.tile([128, T], BF16, tag="A_q", name=f"A_q{p}")
                A_k = pA.tile([128, T], BF16, tag="A_k", name=f"A_k{p}")
                VT = pA.tile([128, T], BF16, tag="VT", name=f"VT{p}")
                vext = pA.tile([128, 16, 2, 65], BF16, tag="vext", name=f"vext{p}")
                nc.vector.memset(vext[:, :, :, 64], 1.0)
                return p, A_q, A_k, VT, vext

            def qkv_steps(st, first=False):
                """Generator of fine-grained QKV emission steps for pair p.
                For the first (un-overlapped) pair, psum->sbuf copies go to
                the otherwise-idle scalar engine so the vector engine's RoPE
                backlog can't stall PSUM bank recycling."""
                p, A_q, A_k, VT, vext = st
                dsts = [A_q, A_k, VT]
                for tci in range(TC):
                    tcs = ds(tci * QC, QC)
                    for mi in range(3):
                        pq = psQ.tile([128, QC], F32, tag="psq", name=f"pq{mi}")
                        for fc in range(FC):
                            nc.tensor.matmul(
                                pq[:],
                                wqk_sb[:, fc, ds(p * 384 + mi * 128, 128)],
                                xT_sb[:, fc, tcs],
                                start=(fc == 0),
                                stop=(fc == FC - 1),
                            )
                        if first:
                            nc.scalar.copy(dsts[mi][:, tcs], pq[:])
                        else:
                            nc.vector.tensor_copy(dsts[mi][:, tcs], pq[:])
                        yield
                    # RoPE on this token chunk (swap dispatch via gpsimd DGE)
                    for A in (A_q, A_k):
                        Bt = pw.tile([128, QC], BF16, tag="Bt")
                        nc.gpsimd.dma_start(Bt[0:32, :], A[32:64, tcs])
                        nc.gpsimd.dma_start(Bt[32:64, :], A[0:32, tcs])
                        nc.gpsimd.dma_start(Bt[64:96, :], A[96:128, tcs])
                        nc.gpsimd.dma_start(Bt[96:128, :], A[64:96, tcs])
                        nc.vector.tensor_tensor(A[:, tcs], A[:, tcs],
                                                cosr[:, tcs], ALU.mult)
                        nc.vector.tensor_tensor(Bt[:], Bt[:], sinr[:, tcs], ALU.mult)
                        nc.vector.tensor_tensor(A[:, tcs], A[:, tcs], Bt[:], ALU.add)
                        yield
                    # V transposes per half as soon as the half is complete,
                    # so attention's first V matmul isn't gated on a long
                    # transpose + re-stride chain at the pair seam.
                    if tci in (1, 3):
                        t0 = (tci - 1) * QC
                        for h in range(2):
                            vtr = pw.tile([128, 8, 64], BF16, tag="vtr",
                                          name=f"vtr{h}")
                            nc.sync.dma_start_transpose(
                                vtr[:], VT[ts(h, 64), ds(t0, 2 * QC)]
                            )
                            nc.vector.tensor_copy(
                                vext[:, ds(8 * (tci // 2), 8), h, 0:64], vtr[:]
                            )
                            yield

            def proj_steps(qj):
                for mt in range(4 * qj, 4 * qj + 4):
                    for nj in range(2):
                        pp = psQ.tile([128, 512], F32, tag="psq", name="pp")
                        for p in range(NP):
                            nc.tensor.matmul(
                                pp[:],
                                outTs[p][:, ts(mt, 128)],
                                wp_sb[:, p, ts(nj, 512)],
                                start=(p == 0),
                                stop=(p == NP - 1),
                            )
                        ob = pw.tile([128, 512], F32, tag="ob")
                        nc.vector.tensor_copy(ob[:], pp[:])
                        nc.sync.dma_start(
                            out_d[ds(mt * 128, 128), ts(nj, 512)], ob[:]
                        )
                        yield

            def emit_attn(st, fillq, last):
                """Attention for pair p. `fillq` is a list of generators of
                filler steps (QKV of the next pair, or the projection of
                completed token chunks), drained between k-blocks to keep
                the PE stream dense."""
                p, A_q, A_k, VT, vext = st

                def pull(n=1):
                    for _ in range(n):
                        while fillq:
                            try:
                                next(fillq[0])
                                break
                            except StopIteration:
                                fillq.pop(0)

                outT = pOT.tile([128, T], BF16, tag="outT", name=f"outT{p}")
                outTs[p] = outT
                for qj in range(NQJ):
                    q0 = qj * QC
                    qw = ds(q0, QC)
                    qkp = pw.tile([128, QC], BF16, tag="qkp")
                    nc.vector.tensor_tensor(qkp[:], A_q[:, qw], A_k[:, qw],
                                            ALU.mult)
                    pd = psQ.tile([2, QC], F32, tag="psq", name="pd")
                    nc.tensor.matmul(pd[:], esel_sb[:], qkp[:], start=True,
                                     stop=True)
                    de = pw.tile([2, QC], F32R, tag="de")
                    nc.scalar.activation(de[:], pd[:], AF.Exp, scale=0.125)

                    po = [
                        psO.tile([65, QC], F32, tag="po", name=f"po{h}")
                        for h in range(2)
                    ]
                    nkc = 4 * qj + 4
                    pts = {}

                    def emit_v(kc):
                        o = kc - 4 * qj
                        c0 = 128 * o if o > 0 else 0
                        pt = pts.pop(kc)
                        for h in range(2):
                            nc.tensor.matmul(
                                po[h][:, c0:QC],
                                vext[:, kc, h, :],
                                pt[:, h, c0:QC],
                                start=(kc == 0),
                                stop=False,
                            )

                    for kc in range(nkc):
                        o = kc - 4 * qj
                        c0 = 128 * o if o > 0 else 0
                        # both heads' S^T blocks into one 2-bank psum tile
                        pst = psS.tile([128, 2, QC], F32, tag="pst")
                        for h in range(2):
                            r0 = 64 * h
                            nc.tensor.matmul(
                                pst[:, h, c0:QC],
                                A_k[r0 : r0 + 64, ts(kc, 128)],
                                A_q[r0 : r0 + 64, ds(q0 + c0, QC - c0)],
                                start=True,
                                stop=True,
                            )
                        # one exp for both heads
                        pt = ppt.tile([128, 2, QC], BF16, tag="pt")
                        nc.scalar.activation(
                            pt[:, :, c0:QC], pst[:, :, c0:QC], AF.Exp,
                            scale=0.125,
                        )
                        if o >= 0:
                            for h in range(2):
                                nc.vector.tensor_tensor(
                                    pt[:, h, ds(c0, 128)],
                                    pt[:, h, ds(c0, 128)],
                                    strictu[:],
                                    ALU.mult,
                                )
                        pts[kc] = pt
                        if kc >= VLAG:
                            emit_v(kc - VLAG)
                        if kc % 2 == 1:
                            pull()
                    for kc in range(max(0, nkc - VLAG), nkc):
                        emit_v(kc)
                    for h in range(2):
                        nc.tensor.matmul(
                            po[h][:], lsel_sb[:, h, :], de[:],
                            start=False, stop=True,
                        )
                        dnr = pw.tile([65, QC], F32R, tag="dnr")
                        nc.vector.tensor_copy(dnr[64:65, :], po[h][64:65, :])
                        pb = psQ.tile([64, QC], F32, tag="psq", name="pb")
                        nc.tensor.matmul(
                            pb[:], onesf_sb[64:65, :], dnr[64:65, :],
                            start=True, stop=True,
                        )
                        bc = pw.tile([64, QC], F32, tag="bc")
                        nc.vector.reciprocal_approx_fast(bc[:], pb[:])
                        if h == 0:
                            nc.vector.tensor_tensor(
                                outT[0:64, qw], po[h][0:64, :], bc[:], ALU.mult
                            )
                        else:
                            oT1 = pw.tile([64, QC], BF16, tag="oT1")
                            nc.vector.tensor_tensor(
                                oT1[:], po[h][0:64, :], bc[:], ALU.mult
                            )
                            nc.gpsimd.dma_start(outT[64:128, qw], oT1[:])
                    pull(2)
                    if last:
                        fillq.append(proj_steps(qj))
                # drain any remaining filler steps
                while fillq:
                    for _ in fillq.pop(0):
                        pass

            # Pair 0: emit QKV through token chunk 1 (q-chunks 0-1 of
            # attention only need those), then start its attention with the
            # rest of its own QKV as the first filler stream.
            st = qkv_alloc(0)
            g0 = qkv_steps(st, first=True)
            for _ in range(12):
                next(g0)
            for p in range(NP):
                cur = st
                fillq = []
                if p == 0:
                    fillq.append(g0)
                if p + 1 < NP:
                    st = qkv_alloc(p + 1)
                    fillq.append(qkv_steps(st))
                emit_attn(cur, fillq, last=(p == NP - 1))

    nc.finalize()
    return nc


def _host_inputs(x, cos, sin, W_qkv, W_proj):
    """Build per-core input maps. Core c -> batch c//2, head-group c%2."""
    x = np.asarray(x, dtype=np.float32)
    cos = np.asarray(cos, dtype=np.float32)
    sin = np.asarray(sin, dtype=np.float32)
    W_qkv = np.asarray(W_qkv, dtype=np.float32)
    W_proj = np.asarray(W_proj, dtype=np.float32)

    cosT = np.ascontiguousarray(cos[0, 0].T)  # [32, T]
    sinT = np.ascontiguousarray(sin[0, 0].T)
    cosr = np.tile(cosT, (4, 1)).astype(BF)  # [128, T]
    sinr = np.concatenate([-sinT, sinT, -sinT, sinT], axis=0).astype(BF)

    esel = np.zeros((128, 2), BF)
    esel[0:64, 0] = 1.0
    esel[64:128, 1] = 1.0
    lsel = np.zeros((2, 2, 65), np.float32)
    lsel[0, 0, 64] = 1.0
    lsel[1, 1, 64] = 1.0
    onesf = np.ones((1, 64), np.float32)
    strictu = np.triu(np.ones((128, 128), np.float32), 1).astype(BF)

    xTb = [
        np.ascontiguousarray(x[b].T).astype(BF) for b in range(B)
    ]  # [D, T] per batch

    in_maps = []
    for c in range(NCORES):
        b, gg = c // 2, c % 2
        heads = [8 * gg + i for i in range(8)]
        cols = []
        for pr in range(NP):
            h0, h1 = heads[2 * pr], heads[2 * pr + 1]
            for base in (0, D, 2 * D):  # q, k, v row blocks of W_qkv
                cols.append(W_qkv[base + 64 * h0 : base + 64 * h0 + 64])
                cols.append(W_qkv[base + 64 * h1 : base + 64 * h1 + 64])
        wqkT = np.ascontiguousarray(np.concatenate(cols, axis=0).T).astype(BF)
        featc = np.concatenate(
            [np.arange(64 * h, 64 * h + 64) for h in heads]
        )
        wpT = np.ascontiguousarray(W_proj[:, featc].T).astype(BF)  # [512, D]
        in_maps.append(
            {
                "xT": xTb[b],
                "cosr": cosr,
                "sinr": sinr,
                "wqkT": wqkT,
                "wpT": wpT,
                "esel": esel,
                "lsel": lsel,
                "onesf": onesf,
                "strictu": strictu,
            }
        )
    return in_maps


_NC_CACHE = {}


def _get_nc():
    if "nc" not in _NC_CACHE:
        _NC_CACHE["nc"] = _build()
    return _NC_CACHE["nc"]


def kernel(x, cos, sin, W_qkv, W_proj, _trace=False, _trace_cores=None):
    from concourse import bass_utils

    nc = _get_nc()
    in_maps = _host_inputs(x, cos, sin, W_qkv, W_proj)
    res = bass_utils.run_bass_kernel_spmd(
        nc,
        in_maps,
        core_ids=list(range(NCORES)),
        trace=_trace,
        trace_cores=_trace_cores,
    )
    out = np.zeros((B, T, D), np.float32)
    for c, r in enumerate(res.results):
        out[c // 2] += r["outp"]
    kernel.last_results = res
    return out
